# revision 73
# baseline (speedup 1.0000x reference)
"""DividedAttentionSublayer on 8 TRN2 NeuronCores.

Sharding: data-parallel over batch (B=8 -> 1 batch element per core),
weights / pos_emb replicated. Per core the attention runs in a
transposed layout (k on partitions, q on free dim). All matmul inputs
are bf16 (host-converted). The relative-position band is fused directly
into the logits PSUM: a skewed-stride DRAM re-read (rel-shift trick)
produces the band in [q, k] order, and a regular matmul against the
identity both transposes it and accumulates it into the logits psum
before a single exp. Clamped tails (|k-q| > 128) enter through two
augmented q rows whose weights (ep2^T Wq / scale) are folded into the
projection host-side; the V bias is folded into the output bias
(bo' = bo + Wo @ bv).

Schedule: the kernel is software-pipelined around the Activation
engine's exp stream (the per-head floor). AV runs flipped (attn
stationary, V moving -> out [q, dh+1] with the denominator as column
64 via a ones-column in V), so softmax division is a per-partition
reciprocal+scale, and comb returns to [dh, q] via a DRAM-strided
transpose (the XBAR dma transpose is broken on this HW). The K
projection (tiles 1..7) and the whole V projection are split into
small units and emitted as per-head-boundary filler inside the
attention loop so the PE array stays busy during exp waits; head 0
pipelines its own V units against its QK steps, heads 13-15 pre-run
the first 7 contraction steps of the output projection (opre), and
heads 14/15 transpose comb via identity matmuls to dodge the DRAM
round-trip latency ahead of the output projection. A burst of junk
matmuls at t=0 burns the PE p-state ramp during the initial DMA wait
so real matmuls start at full clock.
"""
import contextlib
import sys

sys.path.insert(0, "/opt/trn_rl_repo")

import numpy as np
import ml_dtypes
import concourse.bass as bass
import concourse.mybir as mybir
from concourse import bacc
from concourse.tile import TileContext
from concourse.bass import AP
from concourse.bass_utils import run_bass_kernel_spmd

F32 = mybir.dt.float32
BF16 = mybir.dt.bfloat16
FP8 = mybir.dt.float8e4
DR = mybir.MatmulPerfMode.DoubleRow
EXP = mybir.ActivationFunctionType.Exp

B, L, D = 8, 1024, 1024
H, DH = 16, 64
NT = L // 128
SCALE = float(np.sqrt(D / H))
JW = 257           # 2*128 + 1 relative positions
PW = 513           # per-q-tile dpad chunk: 128 pad | 257 mid | 128 pad
PXW = NT * PW      # 4104
BW = 384           # gathered band width per q-tile (3 k-tiles)

_NC = None
DEBUG = False


def _build():
    nc = bacc.Bacc(None, target_bir_lowering=False)

    xq = nc.dram_tensor("xq", [D, L], BF16, kind="ExternalInput")
    xk = nc.dram_tensor("xk", [D, L], BF16, kind="ExternalInput")
    xv = nc.dram_tensor("xv", [D, L], BF16, kind="ExternalInput")
    wq = nc.dram_tensor("wq", [D, D], BF16, kind="ExternalInput")
    wk = nc.dram_tensor("wk", [D, D], BF16, kind="ExternalInput")
    wv = nc.dram_tensor("wv", [D, D], BF16, kind="ExternalInput")
    wo = nc.dram_tensor("wo", [D, D], BF16, kind="ExternalInput")
    wqa = nc.dram_tensor("wqa", [D, 2 * H], BF16, kind="ExternalInput")
    bqc = nc.dram_tensor("bqc", [128, NT], F32, kind="ExternalInput")
    bkc = nc.dram_tensor("bkc", [128, NT], F32, kind="ExternalInput")
    boc = nc.dram_tensor("boc", [128, NT], F32, kind="ExternalInput")
    bqac = nc.dram_tensor("bqac", [2 * H, 1], F32, kind="ExternalInput")
    mkb = nc.dram_tensor("mkb", [128, NT], F32, kind="ExternalInput")
    ept = nc.dram_tensor("ept", [DH, JW], BF16, kind="ExternalInput")
    idn = nc.dram_tensor("idn", [128, 128], BF16, kind="ExternalInput")
    outt = nc.dram_tensor("outt", [D, L], BF16, kind="ExternalOutput")
    if DEBUG:
        dbg_q = nc.dram_tensor("dbg_q", [66, L], BF16, kind="ExternalOutput")
        dbg_k = nc.dram_tensor("dbg_k", [66, L], BF16, kind="ExternalOutput")
        dbg_a = nc.dram_tensor("dbg_a", [128, L], BF16, kind="ExternalOutput")

    rw = lambda t: t[:].rearrange("(c p) l -> p c l", p=128)

    with TileContext(nc) as tc:
        with contextlib.ExitStack() as _st:
            _p = lambda *a, **k: _st.enter_context(tc.tile_pool(*a, **k))
            pp = _p(name="persist", bufs=1)
            pool_qth = _p(name="qth", bufs=16)
            pool_kth = _p(name="kth", bufs=16)
            pool_v = _p(name="vsb", bufs=8)
            pool_ct = _p(name="ct", bufs=8)
            pool_pex = _p(name="pex", bufs=2)
            pool_g = _p(name="gts", bufs=3)
            pool_psA = _p(name="psA", bufs=2, space="PSUM")
            pool_psB = _p(name="psB", bufs=2, space="PSUM")
            pool_xk = _p(name="xk", bufs=2)
            pool_wbk = _p(name="wbk", bufs=2)
            pool_xvp = _p(name="xvp", bufs=2)
            pool_wvb = _p(name="wvb", bufs=1)
            pool_d = _p(name="dram", bufs=5, space="DRAM")
            bq_sb = pp.tile([128, NT], F32, tag="bq")
            bk_sb = pp.tile([128, NT], F32, tag="bk")
            bo_sb = pp.tile([128, NT], F32, tag="bo")
            bqa_sb = pp.tile([2 * H, 1], F32, tag="bqa")
            mk_sb = pp.tile([128, NT], F32, tag="mk")
            ept_b = pp.tile([DH, JW], BF16, tag="eptb")
            ident = pp.tile([128, 128], BF16, tag="ident")
            wqa_sb = pp.tile([128, NT * 2 * H], BF16, tag="wqa")
            wo_first = pp.tile([128, NT * 512], BF16, tag="wof")
            qth = [pool_qth.tile([66, L], BF16, tag="qth", name=f"qth{i}") for i in range(H)]
            kth = [pool_kth.tile([66, L], BF16, tag="kth", name=f"kth{i}") for i in range(H)]
            v_sb = [pool_v.tile([128, H * 65], BF16, tag="v", name=f"vsb{i}") for i in range(NT)]
            ct = [pool_ct.tile([128, L], BF16, tag="ct", name=f"ct{i}") for i in range(NT)]

            # burn the PE p-state ramp during the initial DMA wait: ~3.4us
            # of junk matmuls on memset tiles so real matmuls start at full
            # clock (cost model: 2.4GHz only after 3us of sustained use)
            warm_a = pp.tile([64, 64], BF16, tag="warma")
            warm_b = pp.tile([64, 128], BF16, tag="warmb")
            nc.gpsimd.memset(warm_a[:], 0.0)
            nc.gpsimd.memset(warm_b[:], 0.0)
            warm_ps = pool_psA.tile([64, 128], F32, tag="pav", name="warmps")
            for _ in range(30):
                nc.tensor.matmul(
                    warm_ps[:], warm_a[:], warm_b[:],
                    start=True, stop=True, skip_group_check=True,
                )

            gtiles = {}

            pex_cur = {}

            def band_prep_step(h, m):
                if m == 0:
                    pex_cur[h] = pool_pex.tile([128, PXW], BF16, tag="pex",
                                               name=f"pex{h}")
                pexh = pex_cur[h]
                pp_ps = pool_psA.tile([128, JW], F32, tag="pav")
                nc.tensor.matmul(
                    pp_ps[:], qth[h][0:64, 128 * m : 128 * m + 128], ept_b[:],
                    start=True, stop=True,
                )
                nc.vector.tensor_copy(
                    pexh[:, PW * m + 128 : PW * m + 128 + JW], pp_ps[:]
                )
                # both clamp pads in one op: cols [0,128) <- P col 0,
                # cols [385,513) <- P col 256
                nc.gpsimd.tensor_copy(
                    AP(pexh.tensor, pexh.offset + PW * m,
                       [[PXW, 128], [385, 2], [1, 128]]),
                    AP(pexh.tensor, pexh.offset + PW * m + 128,
                       [[PXW, 128], [256, 2], [0, 128]]),
                )

            dpads = {}

            def band_finish(h):
                pexh = pex_cur.pop(h)
                dpad = pool_d.tile([128, PXW], BF16, tag="dpad")
                dpads[h] = dpad
                nc.sync.dma_start(dpad[:], pexh[:])
                gh = pool_g.tile([128, NT * BW], BF16, tag="g", name=f"g{h}")
                nc.sync.dma_start(
                    gh[:].rearrange("p (m j) -> p m j", j=BW),
                    AP(dpad.tensor, dpad.offset + 128,
                       [[PXW - 1, 128], [PW, NT], [1, BW]]),
                )
                gtiles[h] = gh

            def band_prep(h):
                for m in range(NT):
                    band_prep_step(h, m)
                band_finish(h)

            # =========== Q/K/V projections (bf16) ===========
            with (
                tc.tile_pool(name="xin", bufs=2) as pool_x,
                tc.tile_pool(name="win", bufs=2) as pool_w,
                tc.tile_pool(name="pps", bufs=4, space="PSUM") as pool_ps,
            ):
                x_sb = {}

                def load_xh(name, src, lh, first=False, pool=None, tag="x"):
                    # split by contraction halves onto two queues for latency
                    t = (pool or pool_x).tile([128, NT * 512], BF16, tag=tag,
                                              name=f"x_{name}{lh}")
                    tr = t[:].rearrange("p (c l) -> p c l", l=512)
                    sr = rw(src)[:, :, 512 * lh : 512 * lh + 512]
                    if first:
                        nc.sync.dma_start(tr[:, 0:1, :], sr[:, 0:1, :])
                        nc.sync.dma_start(tr[:, 1:4, :], sr[:, 1:4, :])
                    else:
                        nc.sync.dma_start(tr[:, 0:4, :], sr[:, 0:4, :])
                    nc.gpsimd.dma_start(tr[:, 4:NT, :], sr[:, 4:NT, :])
                    x_sb[name, lh] = t

                wbq_cur = {}

                def proj_qk(xname, wsrc, dst, bcol, irange=None):
                    for i in irange if irange is not None else range(NT):
                        if i not in wbq_cur:
                            # paired 256-col loads: 512B runs dodge the 2x
                            # sub-512B DMA latency multiplier
                            j = i - (i % 2)
                            wb = pool_w.tile([128, NT * 256], BF16, tag="wb",
                                             name=f"wb_{xname}{j}")
                            wb3 = wb[:].rearrange("p (c l) -> p c l", l=256)
                            wsr = rw(wsrc)[:, :, 128 * j : 128 * j + 256]
                            if xname == "q" and j == 0:
                                nc.scalar.dma_start(wb3[:, 0:1, :], wsr[:, 0:1, :])
                                nc.scalar.dma_start(wb3[:, 1:NT, :], wsr[:, 1:NT, :])
                            else:
                                nc.scalar.dma_start(wb3, wsr)
                            wbq_cur[i] = (wb, 0)
                            wbq_cur[i + 1] = (wb, 1)
                        wbt, half = wbq_cur[i]
                        wbr_w = wbt[:].rearrange("p (c l) -> p c l", l=256)
                        wbr = wbr_w[:, :, 128 * half : 128 * half + 128]
                        for lh in range(2):
                            xt = x_sb[xname, lh][:].rearrange("p (c l) -> p c l", l=512)
                            ps = pool_ps.tile([128, 512], F32, tag="ps")
                            for c in range(NT):
                                nc.tensor.matmul(
                                    ps[:], wbr[:, c, :], xt[:, c, :],
                                    start=(c == 0), stop=(c == NT - 1),
                                )
                            for half in range(2):
                                hh = 2 * i + half
                                # group-0 columns coincide with flat cols 0:L
                                nc.vector.tensor_scalar_add(
                                    dst[hh][0:64, 512 * lh : 512 * lh + 512],
                                    ps[64 * half : 64 * half + 64, :],
                                    bcol[64 * half : 64 * half + 64, i : i + 1],
                                )

                wbk_cur = {}

                def k_unit(i, lh):
                    if lh == 0:
                        wb = pool_wbk.tile([128, NT * 128], BF16, tag="wbk",
                                           name=f"wbk{i}")
                        nc.gpsimd.dma_start(
                            wb[:].rearrange("p (c l) -> p c l", l=128),
                            rw(wk)[:, :, 128 * i : 128 * i + 128],
                        )
                        wbk_cur[i] = wb
                    wbr = wbk_cur[i][:].rearrange("p (c l) -> p c l", l=128)
                    xt = x_sb["k", lh][:].rearrange("p (c l) -> p c l", l=512)
                    ps = pool_psB.tile([128, 512], F32, tag="vq", name=f"kps{i}_{lh}")
                    for c in range(NT):
                        nc.tensor.matmul(
                            ps[:], wbr[:, c, :], xt[:, c, :],
                            start=(c == 0), stop=(c == NT - 1),
                        )
                    for half in range(2):
                        hh = 2 * i + half
                        nc.vector.tensor_scalar_add(
                            kth[hh][0:64, 512 * lh : 512 * lh + 512],
                            ps[64 * half : 64 * half + 64, :],
                            bk_sb[64 * half : 64 * half + 64, i : i + 1],
                        )

                wvb_cur = {}

                def v_unit(vb, lt):
                    if vb not in wvb_cur:
                        wvb = pool_wvb.tile([128, NT * 256], BF16, tag="wvb",
                                            name=f"wvb{vb}")
                        nc.gpsimd.dma_start(
                            wvb[:].rearrange("p (c l) -> p c l", l=256),
                            rw(wv)[:, :, 256 * vb : 256 * vb + 256],
                        )
                        wvb_cur.clear()
                        wvb_cur[vb] = wvb
                    wvbr = wvb_cur[vb][:].rearrange("p (c l) -> p c l", l=256)
                    xvt = x_sb["v", lt // 4][:].rearrange("p (c l) -> p c l", l=512)
                    loc = 128 * (lt % 4)
                    ps = pool_psB.tile([128, 256], F32, tag="vq",
                                       name=f"vps{vb}_{lt}")
                    for c in range(NT):
                        nc.tensor.matmul(
                            ps[:],
                            xvt[:, c, loc : loc + 128],
                            wvbr[:, c, :],
                            start=(c == 0),
                            stop=(c == NT - 1),
                        )
                    nc.vector.tensor_copy(
                        v_sb[lt][:].rearrange("p (h c) -> p h c", c=65)[
                            :, 4 * vb : 4 * vb + 4, 0:64
                        ],
                        ps[:].rearrange("p (a b) -> p a b", a=4),
                    )

                load_xh("q", xq, 0, first=True)
                nc.sync.dma_start(bq_sb[:], bqc[:])
                load_xh("q", xq, 1)
                for t, src in ((bk_sb, bkc), (bo_sb, boc),
                               (bqa_sb, bqac), (mk_sb, mkb), (ept_b, ept),
                               (ident, idn)):
                    nc.sync.dma_start(t[:], src[:])
                load_xh("k", xk, 0, pool=pool_xk, tag="xk")

                proj_qk("q", wq, qth, bq_sb, irange=[0])
                # heads 0/1 band prep as early as possible: its DRAM
                # round-trip is on head 0's critical path
                for h in range(2):
                    band_prep(h)
                proj_qk("q", wq, qth, bq_sb, irange=range(1, NT))
                # K projection tile 0 as early as possible (head 0 needs it)
                load_xh("k", xk, 1, pool=pool_xk, tag="xk")
                k_unit(0, 0)
                k_unit(0, 1)
                # augmented q rows, flipped: out [q,32] per q-tile, XBAR
                # transpose to [32, q], bias added per-partition afterwards
                nc.scalar.dma_start(
                    wqa_sb[:].rearrange("p (c m) -> p c m", m=2 * H),
                    wqa[:].rearrange("(c p) m -> p c m", p=128),
                )
                wa = wqa_sb[:].rearrange("p (c m) -> p c m", m=2 * H)
                qflat = pp.tile([128, NT * 2 * H], BF16, tag="qflat")
                qfr = qflat[:].rearrange("p (qt m) -> p qt m", m=2 * H)
                for qt in range(NT):
                    xt = x_sb["q", qt // 4][:].rearrange("p (c l) -> p c l", l=512)
                    loc = 128 * (qt % 4)
                    psq = pool_ps.tile([128, 2 * H], F32, tag="ps", name=f"psq{qt}")
                    for c in range(NT):
                        nc.tensor.matmul(
                            psq[:], xt[:, c, loc : loc + 128], wa[:, c, :],
                            start=(c == 0), stop=(c == NT - 1),
                        )
                    nc.vector.tensor_copy(qfr[:, qt, :], psq[:])
                qaug = pp.tile([2 * H, L], BF16, tag="qaug")
                dqa = pool_d.tile([L, 2 * H], BF16, tag="dqa")
                nc.scalar.dma_start(
                    dqa[:].rearrange("(qt p) m -> p qt m", p=128),
                    qflat[:].rearrange("p (qt m) -> p qt m", m=2 * H),
                )
                nc.scalar.dma_start(qaug[:], dqa[:].rearrange("q m -> m q"))
                nc.vector.tensor_scalar_add(qaug[:], qaug[:], bqa_sb[:, 0:1])
                for h in range(H):
                    eng = nc.scalar if h < 4 else nc.gpsimd
                    eng.dma_start(
                        qth[h][64:66, :], qaug[2 * h : 2 * h + 2, :]
                    )
                # ones rows for clamp-tail aug (kth) and denominator (v_sb);
                # emitted late so they don't head-block the Pool queue's DMAs
                for h in range(H):
                    nc.gpsimd.memset(kth[h][64:66, :], 1.0)
                for lt in range(NT):
                    nc.gpsimd.memset(
                        v_sb[lt][:].rearrange("p (h c) -> p h c", c=65)[:, :, 64:65],
                        1.0,
                    )

                load_xh("v", xv, 0, pool=pool_xvp, tag="xv")
                load_xh("v", xv, 1, pool=pool_xvp, tag="xv")

            # =========== attention ===========
            with (
                tc.tile_pool(name="attn", bufs=9) as pool_attn,
                tc.tile_pool(name="scratch", bufs=2) as pool_s,
                tc.tile_pool(name="psL", bufs=2, space="PSUM") as pool_psL,
            ):
                LOOK = 2
                opre = []
                # filler schedule: K unit (i, lh) due before head 2i; V unit
                # (vb, lt) due before AV of head 4vb. Head 0 pipelines V(0, n)
                # inline against its own QK(n) steps.
                FILLER = {
                    1: [("k", 1, 0), ("k", 1, 1)],
                    2: [("v", 1, 0), ("v", 1, 1), ("v", 1, 2), ("v", 1, 3)],
                    3: [("v", 1, 4), ("v", 1, 5), ("v", 1, 6), ("v", 1, 7),
                        ("k", 2, 0), ("k", 2, 1)],
                    4: [("k", 3, 0)],
                    5: [("k", 3, 1)],
                    6: [("v", 2, 0), ("v", 2, 1), ("v", 2, 2), ("v", 2, 3)],
                    7: [("v", 2, 4), ("v", 2, 5), ("v", 2, 6), ("v", 2, 7),
                        ("k", 4, 0), ("k", 4, 1)],
                    8: [("k", 5, 0)],
                    9: [("k", 5, 1), ("k", 6, 0)],
                    10: [("v", 3, 0), ("v", 3, 1), ("v", 3, 2), ("v", 3, 3),
                         ("k", 7, 0)],
                    11: [("v", 3, 4), ("v", 3, 5), ("v", 3, 6), ("v", 3, 7),
                         ("k", 6, 1)],
                    12: [("k", 7, 1)],
                }

                def run_filler(u):
                    if u[0] == "k":
                        k_unit(u[1], u[2])
                    else:
                        v_unit(u[1], u[2])

                for h in range(H):
                    q = qth[h]
                    k = kth[h]
                    gh = gtiles.pop(h)
                    gr = gh[:].rearrange("p (m j) -> p m j", j=BW)

                    attn = []
                    for n in range(NT):
                        pl = pool_psL.tile([128, L], F32, tag="pl")
                        b0, b1 = max(n - 1, 0), min(n + 2, NT)
                        # far spans with clamp-tail aug rows (no g dependence)
                        spans = []
                        if 128 * (n + 2) < L:
                            spans.append((128 * (n + 2), L, 65))
                        if n - 1 > 0:
                            spans.append((0, 128 * (n - 1), 66))
                        for s0, s1, kk in spans:
                            c0 = s0
                            while c0 < s1:
                                c1 = min(s1, (c0 // 512 + 1) * 512)
                                nc.tensor.matmul(
                                    pl[:, c0:c1],
                                    k[0:kk, 128 * n : 128 * n + 128],
                                    q[0:kk, c0:c1],
                                    start=True, stop=True,
                                    skip_group_check=True,
                                )
                                c0 = c1
                        # band: QK (start) in <=512 chunks, then the
                        # transposed pos band accumulated via identity matmul
                        c0 = 128 * b0
                        while c0 < 128 * b1:
                            c1 = min(128 * b1, (c0 // 512 + 1) * 512)
                            nc.tensor.matmul(
                                pl[:, c0:c1],
                                k[0:64, 128 * n : 128 * n + 128],
                                q[0:64, c0:c1],
                                start=True, stop=False,
                                skip_group_check=True,
                            )
                            c0 = c1
                        for m in range(b0, b1):
                            jb = n - m + 1
                            nc.tensor.matmul(
                                pl[:, 128 * m : 128 * m + 128],
                                gr[:, m, 128 * jb : 128 * jb + 128],
                                ident[:],
                                start=False, stop=True,
                                skip_group_check=True,
                            )
                        at = pool_attn.tile([128, L], BF16, tag="at")
                        nc.scalar.activation(at[:], pl[:], EXP, bias=mk_sb[:, n : n + 1])
                        attn.append(at)
                        if DEBUG and h == 0 and n == 4:
                            nc.sync.dma_start(dbg_a[:], at[:])
                            nc.sync.dma_start(dbg_q[:], qth[0][:])
                            nc.sync.dma_start(dbg_k[:], kth[0][:])
                        if h + LOOK < H:
                            band_prep_step(h + LOOK, n)
                        if h == 0:
                            v_unit(0, n)

                    if h + LOOK < H:
                        band_finish(h + LOOK)
                    if h == 9:
                        nc.sync.dma_start(
                            wo_first[:].rearrange("p (c l) -> p c l", l=512),
                            rw(wo)[:, :, 0:512],
                        )
                    wf0 = wo_first[:].rearrange("p (c l) -> p c l", l=512)

                    def opre_steps(lh0, cs):
                        for c in cs:
                            nc.tensor.matmul(
                                opre[lh0][:],
                                wf0[:, c, 0:128],
                                ct[c][:, 512 * lh0 : 512 * lh0 + 512],
                                start=(c == 0), stop=False,
                                skip_group_check=True,
                            )

                    if h == 13:
                        opre.append(pool_psB.tile([128, 512], F32, tag="vq",
                                                  name="opre0"))
                        opre_steps(0, range(6))
                    elif h == 14:
                        opre.append(pool_psB.tile([128, 512], F32, tag="vq",
                                                  name="opre1"))
                        opre_steps(1, range(3))
                        opre_steps(0, [6])
                    elif h == 15:
                        opre_steps(1, [3, 4, 5, 6])
                    # boundary fillers land in the exp-wait gap ahead of AV
                    for u in FILLER.get(h, []):
                        run_filler(u)
                    # AV flipped: attn stationary, V moving -> out [q, dh+1];
                    # col 64 is the softmax denominator (ones col of v_sb)
                    ctq = pool_s.tile([128, 512], BF16, tag="ctq", name=f"ctq{h}")
                    for qt in range(NT):
                        pav = pool_psA.tile([128, 65], F32, tag="pav")
                        for n in range(NT):
                            nc.tensor.matmul(
                                pav[:],
                                attn[n][:, 128 * qt : 128 * qt + 128],
                                v_sb[n][:, 65 * h : 65 * h + 65],
                                start=(n == 0),
                                stop=(n == NT - 1),
                            )
                        rec = pool_s.tile([128, 1], F32, tag="rec")
                        nc.vector.reciprocal(rec[:], pav[:, 64:65])
                        nc.vector.tensor_scalar_mul(
                            ctq[:, 64 * qt : 64 * qt + 64], pav[:, 0:64], rec[:, 0:1]
                        )
                    if h >= 14:
                        # endgame: transpose via ident matmul + copy, skipping
                        # the DRAM round-trip latency ahead of the output proj
                        for qt in range(NT):
                            ptr = pool_psA.tile([64, 128], F32, tag="pav",
                                                name=f"ptr{h}_{qt}")
                            nc.tensor.matmul(
                                ptr[:], ctq[:, 64 * qt : 64 * qt + 64], ident[:],
                                start=True, stop=True, skip_group_check=True,
                            )
                            nc.vector.tensor_copy(
                                ct[h // 2][
                                    64 * (h % 2) : 64 * (h % 2) + 64,
                                    128 * qt : 128 * qt + 128,
                                ],
                                ptr[:],
                            )
                    else:
                        # transpose [q, dh] -> [dh, q] via DRAM (pure strides)
                        dcth = pool_d.tile([L, DH], BF16, tag="dct",
                                           name=f"dct{h}")
                        nc.sync.dma_start(
                            dcth[:].rearrange("(qt p) d -> p qt d", p=128),
                            ctq[:].rearrange("p (qt d) -> p qt d", d=DH),
                        )
                        nc.sync.dma_start(
                            ct[h // 2][64 * (h % 2) : 64 * (h % 2) + 64, :],
                            dcth[:].rearrange("q d -> d q"),
                        )

            # =========== output projection ===========
            with (
                tc.tile_pool(name="wout", bufs=1) as pool_wo,
                tc.tile_pool(name="oo", bufs=4) as pool_o,
                tc.tile_pool(name="ops", bufs=4, space="PSUM") as pool_ops,
            ):
                wot = pool_wo.tile([128, NT * D], BF16, tag="wo")
                wr = wot[:].rearrange("p (c l) -> p c l", l=D)
                wf = wo_first[:].rearrange("p (c l) -> p c l", l=512)
                sr = rw(wo)
                nc.sync.dma_start(wr[:, :, 512:768], sr[:, :, 512:768])
                nc.sync.dma_start(wr[:, :, 768:D], sr[:, :, 768:D])
                for i in range(NT):
                    ot = pool_o.tile([128, L], BF16, tag="ot")
                    for lh in range(2):
                        if i == 0:
                            ps = opre[lh]
                            nc.tensor.matmul(
                                ps[:],
                                wf[:, NT - 1, 0:128],
                                ct[NT - 1][:, 512 * lh : 512 * lh + 512],
                                start=False, stop=True,
                                skip_group_check=True,
                            )
                        else:
                            ps = pool_ops.tile([128, 512], F32, tag="ps")
                            wsrc, wcol = (wf, 128 * i) if i < 4 else (wr, 128 * i)
                            for c in range(NT):
                                nc.tensor.matmul(
                                    ps[:],
                                    wsrc[:, c, wcol : wcol + 128],
                                    ct[c][:, 512 * lh : 512 * lh + 512],
                                    start=(c == 0),
                                    stop=(c == NT - 1),
                                )
                        nc.vector.tensor_scalar_add(
                            ot[:, 512 * lh : 512 * lh + 512], ps[:], bo_sb[:, i : i + 1]
                        )
                        nc.sync.dma_start(
                            outt[128 * i : 128 * i + 128, 512 * lh : 512 * lh + 512],
                            ot[:, 512 * lh : 512 * lh + 512],
                        )

    nc.compile()
    return nc


def _get_nc():
    global _NC
    if _NC is None:
        _NC = _build()
    return _NC


def _prep_shared(Wq, bq, Wk, bk, Wv, bv, Wo, bo, pos_emb):
    bf = ml_dtypes.bfloat16
    Wq = np.asarray(Wq, np.float32)
    Wk = np.asarray(Wk, np.float32)
    Wv = np.asarray(Wv, np.float32)
    Wo = np.asarray(Wo, np.float32)
    bq = np.asarray(bq, np.float32)
    bk = np.asarray(bk, np.float32)
    bv = np.asarray(bv, np.float32)
    bo = np.asarray(bo, np.float32)
    ep = np.asarray(pos_emb, np.float32)

    wq_arr = np.ascontiguousarray(Wq.T / SCALE)
    wk_arr = np.ascontiguousarray(Wk.T)
    wv_arr = np.ascontiguousarray(Wv.T)
    wo_arr = np.ascontiguousarray(Wo.T)

    ep2 = np.stack([ep[0], ep[2 * 128] - ep[0]], axis=1)       # [64, 2]
    Wq3 = (Wq / SCALE).reshape(H, DH, D)
    # wqa[:, 2h+j] = sum_r ep2[r, j] * Wq3[h, r, :]
    wqa_arr = np.ascontiguousarray(
        np.einsum("rj,hrd->dhj", ep2, Wq3).reshape(D, 2 * H)
    )
    bq3 = (bq / SCALE).reshape(H, DH)
    bqa_arr = np.ascontiguousarray(np.einsum("rj,hr->hj", ep2, bq3).reshape(2 * H, 1))

    bo2 = bo + Wo @ bv
    return {
        "wq": wq_arr.astype(bf), "wk": wk_arr.astype(bf),
        "wv": wv_arr.astype(bf), "wo": wo_arr.astype(bf),
        "wqa": wqa_arr.astype(bf),
        "bqc": np.ascontiguousarray((bq / SCALE).reshape(NT, 128).T),
        "bkc": np.ascontiguousarray(bk.reshape(NT, 128).T),
        "boc": np.ascontiguousarray(bo2.reshape(NT, 128).T),
        "bqac": np.ascontiguousarray(bqa_arr),
        "ept": np.ascontiguousarray(ep.T).astype(bf),
        "idn": np.eye(128, dtype=np.float32).astype(bf),
    }


def _per_core_inputs(shared, inputs, b):
    bf = ml_dtypes.bfloat16
    mrow = np.asarray(inputs["mask"])[b].reshape(L).astype(bool)
    m = dict(shared)
    m["xq"] = np.ascontiguousarray(np.asarray(inputs["x_q"], np.float32)[b].T).astype(bf)
    m["xk"] = np.ascontiguousarray(np.asarray(inputs["x_k"], np.float32)[b].T).astype(bf)
    m["xv"] = np.ascontiguousarray(np.asarray(inputs["x_v"], np.float32)[b].T).astype(bf)
    m["mkb"] = np.ascontiguousarray(
        np.where(mrow, np.float32(-1e30), np.float32(0.0)).reshape(NT, 128).T
    )
    return m


def kernel(x_q, x_k, x_v, mask, Wq, bq, Wk, bk, Wv, bv, Wo, bo, pos_emb):
    nc = _get_nc()
    shared = _prep_shared(Wq, bq, Wk, bk, Wv, bv, Wo, bo, pos_emb)
    inputs = {"x_q": x_q, "x_k": x_k, "x_v": x_v, "mask": mask}
    in_maps = [_per_core_inputs(shared, inputs, b) for b in range(B)]
    res = run_bass_kernel_spmd(nc, in_maps, core_ids=list(range(B)))
    out = np.empty((B, L, D), np.float32)
    for b in range(B):
        out[b] = np.asarray(res.results[b]["outt"], np.float32).T
    return out



# revision 74
# speedup vs baseline: 1.0134x; 1.0134x over previous
"""DividedAttentionSublayer on 8 TRN2 NeuronCores.

Sharding: data-parallel over batch (B=8 -> 1 batch element per core),
weights / pos_emb replicated. Per core the attention runs in a
transposed layout (k on partitions, q on free dim). All matmul inputs
are bf16 (host-converted). The relative-position band is fused directly
into the logits PSUM: a skewed-stride DRAM re-read (rel-shift trick)
produces the band in [q, k] order, and a regular matmul against the
identity both transposes it and accumulates it into the logits psum
before a single exp. Clamped tails (|k-q| > 128) enter through two
augmented q rows whose weights (ep2^T Wq / scale) are folded into the
projection host-side; the V bias is folded into the output bias
(bo' = bo + Wo @ bv).

Schedule: the kernel is software-pipelined around the Activation
engine's exp stream (the per-head floor). AV runs flipped (attn
stationary, V moving -> out [q, dh+1] with the denominator as column
64 via a ones-column in V), so softmax division is a per-partition
reciprocal+scale, and comb returns to [dh, q] via a DRAM-strided
transpose (the XBAR dma transpose is broken on this HW). The K
projection (tiles 1..7) and the whole V projection are split into
small units and emitted as per-head-boundary filler inside the
attention loop so the PE array stays busy during exp waits; head 0
pipelines its own V units against its QK steps, heads 13-15 pre-run
the first 7 contraction steps of the output projection (opre), and
heads 14/15 transpose comb via identity matmuls to dodge the DRAM
round-trip latency ahead of the output projection. A burst of junk
matmuls at t=0 burns the PE p-state ramp during the initial DMA wait
so real matmuls start at full clock.
"""
import contextlib
import sys

sys.path.insert(0, "/opt/trn_rl_repo")

import numpy as np
import ml_dtypes
import concourse.bass as bass
import concourse.mybir as mybir
from concourse import bacc
from concourse.tile import TileContext
from concourse.bass import AP
from concourse.bass_utils import run_bass_kernel_spmd

F32 = mybir.dt.float32
BF16 = mybir.dt.bfloat16
FP8 = mybir.dt.float8e4
DR = mybir.MatmulPerfMode.DoubleRow
EXP = mybir.ActivationFunctionType.Exp

B, L, D = 8, 1024, 1024
H, DH = 16, 64
NT = L // 128
SCALE = float(np.sqrt(D / H))
JW = 257           # 2*128 + 1 relative positions
PW = 513           # per-q-tile dpad chunk: 128 pad | 257 mid | 128 pad
PXW = NT * PW      # 4104
BW = 384           # gathered band width per q-tile (3 k-tiles)

_NC = None
DEBUG = False


def _build():
    nc = bacc.Bacc(None, target_bir_lowering=False)

    xq = nc.dram_tensor("xq", [D, L], BF16, kind="ExternalInput")
    xk = nc.dram_tensor("xk", [D, L], BF16, kind="ExternalInput")
    xv = nc.dram_tensor("xv", [D, L], BF16, kind="ExternalInput")
    wq = nc.dram_tensor("wq", [D, D], BF16, kind="ExternalInput")
    wk = nc.dram_tensor("wk", [D, D], BF16, kind="ExternalInput")
    wv = nc.dram_tensor("wv", [D, D], BF16, kind="ExternalInput")
    wo = nc.dram_tensor("wo", [D, D], BF16, kind="ExternalInput")
    wqa = nc.dram_tensor("wqa", [D, 2 * H], BF16, kind="ExternalInput")
    bqc = nc.dram_tensor("bqc", [128, NT], F32, kind="ExternalInput")
    bkc = nc.dram_tensor("bkc", [128, NT], F32, kind="ExternalInput")
    boc = nc.dram_tensor("boc", [128, NT], F32, kind="ExternalInput")
    bqac = nc.dram_tensor("bqac", [2 * H, 1], F32, kind="ExternalInput")
    mkb = nc.dram_tensor("mkb", [128, NT], F32, kind="ExternalInput")
    ept = nc.dram_tensor("ept", [DH, JW], BF16, kind="ExternalInput")
    idn = nc.dram_tensor("idn", [128, 128], BF16, kind="ExternalInput")
    outt = nc.dram_tensor("outt", [D, L], BF16, kind="ExternalOutput")
    if DEBUG:
        dbg_q = nc.dram_tensor("dbg_q", [66, L], BF16, kind="ExternalOutput")
        dbg_k = nc.dram_tensor("dbg_k", [66, L], BF16, kind="ExternalOutput")
        dbg_a = nc.dram_tensor("dbg_a", [128, L], BF16, kind="ExternalOutput")

    rw = lambda t: t[:].rearrange("(c p) l -> p c l", p=128)

    with TileContext(nc) as tc:
        with contextlib.ExitStack() as _st:
            _p = lambda *a, **k: _st.enter_context(tc.tile_pool(*a, **k))
            pp = _p(name="persist", bufs=1)
            pool_qth = _p(name="qth", bufs=16)
            pool_kth = _p(name="kth", bufs=16)
            pool_v = _p(name="vsb", bufs=8)
            pool_ct = _p(name="ct", bufs=8)
            pool_pex = _p(name="pex", bufs=2)
            pool_g = _p(name="gts", bufs=3)
            pool_psA = _p(name="psA", bufs=2, space="PSUM")
            pool_psB = _p(name="psB", bufs=2, space="PSUM")
            pool_xk = _p(name="xk", bufs=2)
            pool_wbk = _p(name="wbk", bufs=2)
            pool_xvp = _p(name="xvp", bufs=2)
            pool_wvb = _p(name="wvb", bufs=1)
            pool_d = _p(name="dram", bufs=5, space="DRAM")
            bq_sb = pp.tile([128, NT], F32, tag="bq")
            bk_sb = pp.tile([128, NT], F32, tag="bk")
            bo_sb = pp.tile([128, NT], F32, tag="bo")
            bqa_sb = pp.tile([2 * H, 1], F32, tag="bqa")
            mk_sb = pp.tile([128, NT], F32, tag="mk")
            ept_b = pp.tile([DH, JW], BF16, tag="eptb")
            ident = pp.tile([128, 128], BF16, tag="ident")
            wqa_sb = pp.tile([128, NT * 2 * H], BF16, tag="wqa")
            wo_first = pp.tile([128, NT * 512], BF16, tag="wof")
            qth = [pool_qth.tile([66, L], BF16, tag="qth", name=f"qth{i}") for i in range(H)]
            kth = [pool_kth.tile([66, L], BF16, tag="kth", name=f"kth{i}") for i in range(H)]
            v_sb = [pool_v.tile([128, H * 65], BF16, tag="v", name=f"vsb{i}") for i in range(NT)]
            ct = [pool_ct.tile([128, L], BF16, tag="ct", name=f"ct{i}") for i in range(NT)]

            # burn the PE p-state ramp during the initial DMA wait: ~3.4us
            # of junk matmuls on memset tiles so real matmuls start at full
            # clock (cost model: 2.4GHz only after 3us of sustained use)
            warm_a = pp.tile([64, 64], BF16, tag="warma")
            warm_b = pp.tile([64, 128], BF16, tag="warmb")
            nc.gpsimd.memset(warm_a[:], 0.0)
            nc.gpsimd.memset(warm_b[:], 0.0)
            warm_ps = pool_psA.tile([64, 128], F32, tag="pav", name="warmps")
            for _ in range(30):
                nc.tensor.matmul(
                    warm_ps[:], warm_a[:], warm_b[:],
                    start=True, stop=True, skip_group_check=True,
                )

            gtiles = {}

            pex_cur = {}

            def band_prep_step(h, m):
                if m == 0:
                    pex_cur[h] = pool_pex.tile([128, PXW], BF16, tag="pex",
                                               name=f"pex{h}")
                pexh = pex_cur[h]
                pp_ps = pool_psA.tile([128, JW], F32, tag="pav")
                nc.tensor.matmul(
                    pp_ps[:], qth[h][0:64, 128 * m : 128 * m + 128], ept_b[:],
                    start=True, stop=True,
                )
                nc.vector.tensor_copy(
                    pexh[:, PW * m + 128 : PW * m + 128 + JW], pp_ps[:]
                )
                # both clamp pads in one op: cols [0,128) <- P col 0,
                # cols [385,513) <- P col 256
                nc.gpsimd.tensor_copy(
                    AP(pexh.tensor, pexh.offset + PW * m,
                       [[PXW, 128], [385, 2], [1, 128]]),
                    AP(pexh.tensor, pexh.offset + PW * m + 128,
                       [[PXW, 128], [256, 2], [0, 128]]),
                )

            dpads = {}

            def band_finish(h):
                pexh = pex_cur.pop(h)
                dpad = pool_d.tile([128, PXW], BF16, tag="dpad")
                dpads[h] = dpad
                nc.sync.dma_start(dpad[:], pexh[:])
                gh = pool_g.tile([128, NT * BW], BF16, tag="g", name=f"g{h}")
                nc.sync.dma_start(
                    gh[:].rearrange("p (m j) -> p m j", j=BW),
                    AP(dpad.tensor, dpad.offset + 128,
                       [[PXW - 1, 128], [PW, NT], [1, BW]]),
                )
                gtiles[h] = gh

            def band_prep(h):
                for m in range(NT):
                    band_prep_step(h, m)
                band_finish(h)

            # =========== Q/K/V projections (bf16) ===========
            with (
                tc.tile_pool(name="xin", bufs=2) as pool_x,
                tc.tile_pool(name="win", bufs=2) as pool_w,
                tc.tile_pool(name="pps", bufs=4, space="PSUM") as pool_ps,
            ):
                x_sb = {}

                def load_xh(name, src, lh, first=False, pool=None, tag="x"):
                    # split by contraction halves onto two queues for latency
                    t = (pool or pool_x).tile([128, NT * 512], BF16, tag=tag,
                                              name=f"x_{name}{lh}")
                    tr = t[:].rearrange("p (c l) -> p c l", l=512)
                    sr = rw(src)[:, :, 512 * lh : 512 * lh + 512]
                    if first:
                        nc.sync.dma_start(tr[:, 0:1, :], sr[:, 0:1, :])
                        nc.sync.dma_start(tr[:, 1:4, :], sr[:, 1:4, :])
                    else:
                        nc.sync.dma_start(tr[:, 0:4, :], sr[:, 0:4, :])
                    nc.gpsimd.dma_start(tr[:, 4:NT, :], sr[:, 4:NT, :])
                    x_sb[name, lh] = t

                wbq_cur = {}

                def proj_qk(xname, wsrc, dst, bcol, irange=None):
                    for i in irange if irange is not None else range(NT):
                        if i not in wbq_cur:
                            # paired 256-col loads: 512B runs dodge the 2x
                            # sub-512B DMA latency multiplier
                            j = i - (i % 2)
                            wb = pool_w.tile([128, NT * 256], BF16, tag="wb",
                                             name=f"wb_{xname}{j}")
                            wb3 = wb[:].rearrange("p (c l) -> p c l", l=256)
                            wsr = rw(wsrc)[:, :, 128 * j : 128 * j + 256]
                            if xname == "q" and j == 0:
                                nc.scalar.dma_start(wb3[:, 0:1, :], wsr[:, 0:1, :])
                                nc.scalar.dma_start(wb3[:, 1:NT, :], wsr[:, 1:NT, :])
                            else:
                                nc.scalar.dma_start(wb3, wsr)
                            wbq_cur[i] = (wb, 0)
                            wbq_cur[i + 1] = (wb, 1)
                        wbt, half = wbq_cur[i]
                        wbr_w = wbt[:].rearrange("p (c l) -> p c l", l=256)
                        wbr = wbr_w[:, :, 128 * half : 128 * half + 128]
                        for lh in range(2):
                            xt = x_sb[xname, lh][:].rearrange("p (c l) -> p c l", l=512)
                            ps = pool_ps.tile([128, 512], F32, tag="ps")
                            for c in range(NT):
                                nc.tensor.matmul(
                                    ps[:], wbr[:, c, :], xt[:, c, :],
                                    start=(c == 0), stop=(c == NT - 1),
                                )
                            for half in range(2):
                                hh = 2 * i + half
                                # group-0 columns coincide with flat cols 0:L
                                nc.vector.tensor_scalar_add(
                                    dst[hh][0:64, 512 * lh : 512 * lh + 512],
                                    ps[64 * half : 64 * half + 64, :],
                                    bcol[64 * half : 64 * half + 64, i : i + 1],
                                )

                wbk_cur = {}

                def k_unit(i, lh):
                    if lh == 0:
                        wb = pool_wbk.tile([128, NT * 128], BF16, tag="wbk",
                                           name=f"wbk{i}")
                        nc.sync.dma_start(
                            wb[:].rearrange("p (c l) -> p c l", l=128),
                            rw(wk)[:, :, 128 * i : 128 * i + 128],
                        )
                        wbk_cur[i] = wb
                    wbr = wbk_cur[i][:].rearrange("p (c l) -> p c l", l=128)
                    xt = x_sb["k", lh][:].rearrange("p (c l) -> p c l", l=512)
                    ps = pool_psB.tile([128, 512], F32, tag="vq", name=f"kps{i}_{lh}")
                    for c in range(NT):
                        nc.tensor.matmul(
                            ps[:], wbr[:, c, :], xt[:, c, :],
                            start=(c == 0), stop=(c == NT - 1),
                        )
                    for half in range(2):
                        hh = 2 * i + half
                        nc.vector.tensor_scalar_add(
                            kth[hh][0:64, 512 * lh : 512 * lh + 512],
                            ps[64 * half : 64 * half + 64, :],
                            bk_sb[64 * half : 64 * half + 64, i : i + 1],
                        )

                wvb_cur = {}

                def v_unit(vb, lt):
                    if vb not in wvb_cur:
                        wvb = pool_wvb.tile([128, NT * 256], BF16, tag="wvb",
                                            name=f"wvb{vb}")
                        nc.sync.dma_start(
                            wvb[:].rearrange("p (c l) -> p c l", l=256),
                            rw(wv)[:, :, 256 * vb : 256 * vb + 256],
                        )
                        wvb_cur.clear()
                        wvb_cur[vb] = wvb
                    wvbr = wvb_cur[vb][:].rearrange("p (c l) -> p c l", l=256)
                    xvt = x_sb["v", lt // 4][:].rearrange("p (c l) -> p c l", l=512)
                    loc = 128 * (lt % 4)
                    ps = pool_psB.tile([128, 256], F32, tag="vq",
                                       name=f"vps{vb}_{lt}")
                    for c in range(NT):
                        nc.tensor.matmul(
                            ps[:],
                            xvt[:, c, loc : loc + 128],
                            wvbr[:, c, :],
                            start=(c == 0),
                            stop=(c == NT - 1),
                        )
                    nc.vector.tensor_copy(
                        v_sb[lt][:].rearrange("p (h c) -> p h c", c=65)[
                            :, 4 * vb : 4 * vb + 4, 0:64
                        ],
                        ps[:].rearrange("p (a b) -> p a b", a=4),
                    )

                load_xh("q", xq, 0, first=True)
                nc.sync.dma_start(bq_sb[:], bqc[:])
                load_xh("q", xq, 1)
                for t, src in ((bk_sb, bkc), (bo_sb, boc),
                               (bqa_sb, bqac), (mk_sb, mkb), (ept_b, ept),
                               (ident, idn)):
                    nc.sync.dma_start(t[:], src[:])
                load_xh("k", xk, 0, pool=pool_xk, tag="xk")

                proj_qk("q", wq, qth, bq_sb, irange=[0])
                # heads 0/1 band prep as early as possible: its DRAM
                # round-trip is on head 0's critical path
                for h in range(2):
                    band_prep(h)
                proj_qk("q", wq, qth, bq_sb, irange=range(1, NT))
                # K projection tile 0 as early as possible (head 0 needs it)
                load_xh("k", xk, 1, pool=pool_xk, tag="xk")
                k_unit(0, 0)
                k_unit(0, 1)
                # augmented q rows, flipped: out [q,32] per q-tile, XBAR
                # transpose to [32, q], bias added per-partition afterwards
                nc.scalar.dma_start(
                    wqa_sb[:].rearrange("p (c m) -> p c m", m=2 * H),
                    wqa[:].rearrange("(c p) m -> p c m", p=128),
                )
                wa = wqa_sb[:].rearrange("p (c m) -> p c m", m=2 * H)
                qflat = pp.tile([128, NT * 2 * H], BF16, tag="qflat")
                qfr = qflat[:].rearrange("p (qt m) -> p qt m", m=2 * H)
                for qt in range(NT):
                    xt = x_sb["q", qt // 4][:].rearrange("p (c l) -> p c l", l=512)
                    loc = 128 * (qt % 4)
                    psq = pool_ps.tile([128, 2 * H], F32, tag="ps", name=f"psq{qt}")
                    for c in range(NT):
                        nc.tensor.matmul(
                            psq[:], xt[:, c, loc : loc + 128], wa[:, c, :],
                            start=(c == 0), stop=(c == NT - 1),
                        )
                    nc.vector.tensor_copy(qfr[:, qt, :], psq[:])
                qaug = pp.tile([2 * H, L], BF16, tag="qaug")
                dqa = pool_d.tile([L, 2 * H], BF16, tag="dqa")
                nc.scalar.dma_start(
                    dqa[:].rearrange("(qt p) m -> p qt m", p=128),
                    qflat[:].rearrange("p (qt m) -> p qt m", m=2 * H),
                )
                nc.scalar.dma_start(qaug[:], dqa[:].rearrange("q m -> m q"))
                nc.vector.tensor_scalar_add(qaug[:], qaug[:], bqa_sb[:, 0:1])
                for h in range(H):
                    eng = nc.scalar if h < 4 else nc.gpsimd
                    eng.dma_start(
                        qth[h][64:66, :], qaug[2 * h : 2 * h + 2, :]
                    )
                # ones rows for clamp-tail aug (kth) and denominator (v_sb);
                # emitted late so they don't head-block the Pool queue's DMAs
                for h in range(H):
                    nc.gpsimd.memset(kth[h][64:66, :], 1.0)
                for lt in range(NT):
                    nc.gpsimd.memset(
                        v_sb[lt][:].rearrange("p (h c) -> p h c", c=65)[:, :, 64:65],
                        1.0,
                    )

                load_xh("v", xv, 0, pool=pool_xvp, tag="xv")
                load_xh("v", xv, 1, pool=pool_xvp, tag="xv")

            # =========== attention ===========
            with (
                tc.tile_pool(name="attn", bufs=9) as pool_attn,
                tc.tile_pool(name="scratch", bufs=2) as pool_s,
                tc.tile_pool(name="psL", bufs=2, space="PSUM") as pool_psL,
            ):
                LOOK = 2
                opre = []
                # filler schedule: K unit (i, lh) due before head 2i; V unit
                # (vb, lt) due before AV of head 4vb. Head 0 pipelines V(0, n)
                # inline against its own QK(n) steps.
                FILLER = {
                    1: [("k", 1, 0), ("k", 1, 1)],
                    2: [("v", 1, 0), ("v", 1, 1), ("v", 1, 2), ("v", 1, 3)],
                    3: [("v", 1, 4), ("v", 1, 5), ("v", 1, 6), ("v", 1, 7),
                        ("k", 2, 0), ("k", 2, 1)],
                    4: [("k", 3, 0)],
                    5: [("k", 3, 1)],
                    6: [("v", 2, 0), ("v", 2, 1), ("v", 2, 2), ("v", 2, 3)],
                    7: [("v", 2, 4), ("v", 2, 5), ("v", 2, 6), ("v", 2, 7),
                        ("k", 4, 0), ("k", 4, 1)],
                    8: [("k", 5, 0)],
                    9: [("k", 5, 1), ("k", 6, 0)],
                    10: [("v", 3, 0), ("v", 3, 1), ("v", 3, 2), ("v", 3, 3),
                         ("k", 7, 0)],
                    11: [("v", 3, 4), ("v", 3, 5), ("v", 3, 6), ("v", 3, 7),
                         ("k", 6, 1)],
                    12: [("k", 7, 1)],
                }

                def run_filler(u):
                    if u[0] == "k":
                        k_unit(u[1], u[2])
                    else:
                        v_unit(u[1], u[2])

                for h in range(H):
                    q = qth[h]
                    k = kth[h]
                    gh = gtiles.pop(h)
                    gr = gh[:].rearrange("p (m j) -> p m j", j=BW)

                    attn = []
                    for n in range(NT):
                        pl = pool_psL.tile([128, L], F32, tag="pl")
                        b0, b1 = max(n - 1, 0), min(n + 2, NT)
                        # far spans with clamp-tail aug rows (no g dependence)
                        spans = []
                        if 128 * (n + 2) < L:
                            spans.append((128 * (n + 2), L, 65))
                        if n - 1 > 0:
                            spans.append((0, 128 * (n - 1), 66))
                        for s0, s1, kk in spans:
                            c0 = s0
                            while c0 < s1:
                                c1 = min(s1, (c0 // 512 + 1) * 512)
                                nc.tensor.matmul(
                                    pl[:, c0:c1],
                                    k[0:kk, 128 * n : 128 * n + 128],
                                    q[0:kk, c0:c1],
                                    start=True, stop=True,
                                    skip_group_check=True,
                                )
                                c0 = c1
                        # band: QK (start) in <=512 chunks, then the
                        # transposed pos band accumulated via identity matmul
                        c0 = 128 * b0
                        while c0 < 128 * b1:
                            c1 = min(128 * b1, (c0 // 512 + 1) * 512)
                            nc.tensor.matmul(
                                pl[:, c0:c1],
                                k[0:64, 128 * n : 128 * n + 128],
                                q[0:64, c0:c1],
                                start=True, stop=False,
                                skip_group_check=True,
                            )
                            c0 = c1
                        for m in range(b0, b1):
                            jb = n - m + 1
                            nc.tensor.matmul(
                                pl[:, 128 * m : 128 * m + 128],
                                gr[:, m, 128 * jb : 128 * jb + 128],
                                ident[:],
                                start=False, stop=True,
                                skip_group_check=True,
                            )
                        at = pool_attn.tile([128, L], BF16, tag="at")
                        nc.scalar.activation(at[:], pl[:], EXP, bias=mk_sb[:, n : n + 1])
                        attn.append(at)
                        if DEBUG and h == 0 and n == 4:
                            nc.sync.dma_start(dbg_a[:], at[:])
                            nc.sync.dma_start(dbg_q[:], qth[0][:])
                            nc.sync.dma_start(dbg_k[:], kth[0][:])
                        if h + LOOK < H:
                            band_prep_step(h + LOOK, n)
                        if h == 0:
                            v_unit(0, n)

                    if h + LOOK < H:
                        band_finish(h + LOOK)
                    if h == 9:
                        nc.sync.dma_start(
                            wo_first[:].rearrange("p (c l) -> p c l", l=512),
                            rw(wo)[:, :, 0:512],
                        )
                    wf0 = wo_first[:].rearrange("p (c l) -> p c l", l=512)

                    def opre_steps(lh0, cs):
                        for c in cs:
                            nc.tensor.matmul(
                                opre[lh0][:],
                                wf0[:, c, 0:128],
                                ct[c][:, 512 * lh0 : 512 * lh0 + 512],
                                start=(c == 0), stop=False,
                                skip_group_check=True,
                            )

                    if h == 13:
                        opre.append(pool_psB.tile([128, 512], F32, tag="vq",
                                                  name="opre0"))
                        opre_steps(0, range(6))
                    elif h == 14:
                        opre.append(pool_psB.tile([128, 512], F32, tag="vq",
                                                  name="opre1"))
                        opre_steps(1, range(3))
                        opre_steps(0, [6])
                    elif h == 15:
                        opre_steps(1, [3, 4, 5, 6])
                    # boundary fillers land in the exp-wait gap ahead of AV
                    for u in FILLER.get(h, []):
                        run_filler(u)
                    # AV flipped: attn stationary, V moving -> out [q, dh+1];
                    # col 64 is the softmax denominator (ones col of v_sb)
                    ctq = pool_s.tile([128, 512], BF16, tag="ctq", name=f"ctq{h}")
                    for qt in range(NT):
                        pav = pool_psA.tile([128, 65], F32, tag="pav")
                        for n in range(NT):
                            nc.tensor.matmul(
                                pav[:],
                                attn[n][:, 128 * qt : 128 * qt + 128],
                                v_sb[n][:, 65 * h : 65 * h + 65],
                                start=(n == 0),
                                stop=(n == NT - 1),
                            )
                        rec = pool_s.tile([128, 1], F32, tag="rec")
                        nc.vector.reciprocal(rec[:], pav[:, 64:65])
                        nc.vector.tensor_scalar_mul(
                            ctq[:, 64 * qt : 64 * qt + 64], pav[:, 0:64], rec[:, 0:1]
                        )
                    if h >= 14:
                        # endgame: transpose via ident matmul + copy, skipping
                        # the DRAM round-trip latency ahead of the output proj
                        for qt in range(NT):
                            ptr = pool_psA.tile([64, 128], F32, tag="pav",
                                                name=f"ptr{h}_{qt}")
                            nc.tensor.matmul(
                                ptr[:], ctq[:, 64 * qt : 64 * qt + 64], ident[:],
                                start=True, stop=True, skip_group_check=True,
                            )
                            nc.vector.tensor_copy(
                                ct[h // 2][
                                    64 * (h % 2) : 64 * (h % 2) + 64,
                                    128 * qt : 128 * qt + 128,
                                ],
                                ptr[:],
                            )
                    else:
                        # transpose [q, dh] -> [dh, q] via DRAM (pure strides)
                        dcth = pool_d.tile([L, DH], BF16, tag="dct",
                                           name=f"dct{h}")
                        nc.sync.dma_start(
                            dcth[:].rearrange("(qt p) d -> p qt d", p=128),
                            ctq[:].rearrange("p (qt d) -> p qt d", d=DH),
                        )
                        nc.sync.dma_start(
                            ct[h // 2][64 * (h % 2) : 64 * (h % 2) + 64, :],
                            dcth[:].rearrange("q d -> d q"),
                        )

            # =========== output projection ===========
            with (
                tc.tile_pool(name="wout", bufs=1) as pool_wo,
                tc.tile_pool(name="oo", bufs=4) as pool_o,
                tc.tile_pool(name="ops", bufs=4, space="PSUM") as pool_ops,
            ):
                wot = pool_wo.tile([128, NT * D], BF16, tag="wo")
                wr = wot[:].rearrange("p (c l) -> p c l", l=D)
                wf = wo_first[:].rearrange("p (c l) -> p c l", l=512)
                sr = rw(wo)
                nc.sync.dma_start(wr[:, :, 512:768], sr[:, :, 512:768])
                nc.sync.dma_start(wr[:, :, 768:D], sr[:, :, 768:D])
                for i in range(NT):
                    ot = pool_o.tile([128, L], BF16, tag="ot")
                    for lh in range(2):
                        if i == 0:
                            ps = opre[lh]
                            nc.tensor.matmul(
                                ps[:],
                                wf[:, NT - 1, 0:128],
                                ct[NT - 1][:, 512 * lh : 512 * lh + 512],
                                start=False, stop=True,
                                skip_group_check=True,
                            )
                        else:
                            ps = pool_ops.tile([128, 512], F32, tag="ps")
                            wsrc, wcol = (wf, 128 * i) if i < 4 else (wr, 128 * i)
                            for c in range(NT):
                                nc.tensor.matmul(
                                    ps[:],
                                    wsrc[:, c, wcol : wcol + 128],
                                    ct[c][:, 512 * lh : 512 * lh + 512],
                                    start=(c == 0),
                                    stop=(c == NT - 1),
                                )
                        nc.vector.tensor_scalar_add(
                            ot[:, 512 * lh : 512 * lh + 512], ps[:], bo_sb[:, i : i + 1]
                        )
                        nc.sync.dma_start(
                            outt[128 * i : 128 * i + 128, 512 * lh : 512 * lh + 512],
                            ot[:, 512 * lh : 512 * lh + 512],
                        )

    nc.compile()
    return nc


def _get_nc():
    global _NC
    if _NC is None:
        _NC = _build()
    return _NC


def _prep_shared(Wq, bq, Wk, bk, Wv, bv, Wo, bo, pos_emb):
    bf = ml_dtypes.bfloat16
    Wq = np.asarray(Wq, np.float32)
    Wk = np.asarray(Wk, np.float32)
    Wv = np.asarray(Wv, np.float32)
    Wo = np.asarray(Wo, np.float32)
    bq = np.asarray(bq, np.float32)
    bk = np.asarray(bk, np.float32)
    bv = np.asarray(bv, np.float32)
    bo = np.asarray(bo, np.float32)
    ep = np.asarray(pos_emb, np.float32)

    wq_arr = np.ascontiguousarray(Wq.T / SCALE)
    wk_arr = np.ascontiguousarray(Wk.T)
    wv_arr = np.ascontiguousarray(Wv.T)
    wo_arr = np.ascontiguousarray(Wo.T)

    ep2 = np.stack([ep[0], ep[2 * 128] - ep[0]], axis=1)       # [64, 2]
    Wq3 = (Wq / SCALE).reshape(H, DH, D)
    # wqa[:, 2h+j] = sum_r ep2[r, j] * Wq3[h, r, :]
    wqa_arr = np.ascontiguousarray(
        np.einsum("rj,hrd->dhj", ep2, Wq3).reshape(D, 2 * H)
    )
    bq3 = (bq / SCALE).reshape(H, DH)
    bqa_arr = np.ascontiguousarray(np.einsum("rj,hr->hj", ep2, bq3).reshape(2 * H, 1))

    bo2 = bo + Wo @ bv
    return {
        "wq": wq_arr.astype(bf), "wk": wk_arr.astype(bf),
        "wv": wv_arr.astype(bf), "wo": wo_arr.astype(bf),
        "wqa": wqa_arr.astype(bf),
        "bqc": np.ascontiguousarray((bq / SCALE).reshape(NT, 128).T),
        "bkc": np.ascontiguousarray(bk.reshape(NT, 128).T),
        "boc": np.ascontiguousarray(bo2.reshape(NT, 128).T),
        "bqac": np.ascontiguousarray(bqa_arr),
        "ept": np.ascontiguousarray(ep.T).astype(bf),
        "idn": np.eye(128, dtype=np.float32).astype(bf),
    }


def _per_core_inputs(shared, inputs, b):
    bf = ml_dtypes.bfloat16
    mrow = np.asarray(inputs["mask"])[b].reshape(L).astype(bool)
    m = dict(shared)
    m["xq"] = np.ascontiguousarray(np.asarray(inputs["x_q"], np.float32)[b].T).astype(bf)
    m["xk"] = np.ascontiguousarray(np.asarray(inputs["x_k"], np.float32)[b].T).astype(bf)
    m["xv"] = np.ascontiguousarray(np.asarray(inputs["x_v"], np.float32)[b].T).astype(bf)
    m["mkb"] = np.ascontiguousarray(
        np.where(mrow, np.float32(-1e30), np.float32(0.0)).reshape(NT, 128).T
    )
    return m


def kernel(x_q, x_k, x_v, mask, Wq, bq, Wk, bk, Wv, bv, Wo, bo, pos_emb):
    nc = _get_nc()
    shared = _prep_shared(Wq, bq, Wk, bk, Wv, bv, Wo, bo, pos_emb)
    inputs = {"x_q": x_q, "x_k": x_k, "x_v": x_v, "mask": mask}
    in_maps = [_per_core_inputs(shared, inputs, b) for b in range(B)]
    res = run_bass_kernel_spmd(nc, in_maps, core_ids=list(range(B)))
    out = np.empty((B, L, D), np.float32)
    for b in range(B):
        out[b] = np.asarray(res.results[b]["outt"], np.float32).T
    return out

